# revision 1
# baseline (speedup 1.0000x reference)
"""Multi-head attention (B=2, S=2048, D=1024, H=16) on 8 TRN2 NeuronCores.

Sharding: core c -> (batch b = c//4, head-group g = c%4). Each core computes
the attention output restricted to its batch and its 4 heads (a 256-wide
slice of the model dim), including the row-parallel output projection
partial product. Host sums the 4 partials per batch and adds bo.

Device-side layouts (everything transposed so no on-device transposes are
needed):
  xq/xk/xv  bf16 [1025, 2048]  = x[b].T with a trailing ones row (bias trick)
  wq/wk/wv  bf16 [1025, 256]   = W[g-slice, :].T with trailing bias row
  wo        bf16 [256, 1024]   = Wo[:, g-slice].T
  outT      f32  [1024, 2048]  = (Wo_g @ ctxn_g^T) partial, host transposes

Pipeline per core:
  Q^T,K^T = W x^T            (PE, contraction over model dim, psum accum)
  V       = x^T-stationary   (natural [s, d] layout, +ones column -> Z sums)
  per head: scores^T[k,q] = K_h^T-stationary @ Q_h^T   (psum [128,2048])
            attn = exp(scores/8)                        (ACT, psum->sbuf bf16)
            ctx_aug^T[d+1,q] += V_aug^T-stationary @ attn (psum accum)
            ctxn^T = ctx^T * recip(Z) broadcast          (DVE + gpsimd bcast)
  outT[oc] = wo-stationary @ ctxn^T                      (PE, psum accum)
"""

import numpy as np
import ml_dtypes

from concourse import bacc, tile, mybir
from concourse.bass_utils import run_bass_kernel_spmd

BF16 = mybir.dt.bfloat16
F32 = mybir.dt.float32

S = 2048      # sequence length
D = 1024      # model dim
DG = 256      # per-core head-group width (4 heads x 64)
DK = 64       # head dim
NH = 4        # heads per core
MT = 8        # model-dim contraction tiles (1024 / 128)
QC = 4        # q chunks of 512
KC = 16       # k chunks of 128
N_CORES = 8


def _copy_evict(nc, idx, out_ap, in_ap):
    """Alternate PSUM->SBUF evictions between DVE and ACT to split the load."""
    if idx % 2 == 0:
        nc.vector.tensor_copy(out_ap, in_ap)
    else:
        nc.scalar.copy(out_ap, in_ap)


def _emit(nc, pools, dram):
    persist, xp, wp, wop, attnp, zp, outp, ps, ctxps, smallps = pools
    xq, xk, xv, wq, wk, wv, bT, wo, outT0, outT1 = dram
    HS = S // 2  # 1024-wide half grains

    # persistent tiles for this iteration
    qt = [persist.tile([128, S], BF16, tag=f"qt{i}", name=f"qt{i}") for i in range(2)]
    kt = [persist.tile([128, S], BF16, tag=f"kt{i}", name=f"kt{i}") for i in range(2)]
    ctxn = [persist.tile([128, S], BF16, tag=f"ctxn{i}", name=f"ctxn{i}") for i in range(2)]
    vaug = persist.tile([128, KC, NH, DK + 1], BF16, tag="vaug", name="vaug")
    ones = persist.tile([1, S], BF16, tag="ones", name="ones")

    nc.vector.memset(ones[:], 1.0)
    bt = persist.tile([1, 3 * DG], BF16, tag="bt", name="bt")
    # ones columns of V_aug (softmax denominator accumulates here)
    nc.vector.memset(vaug[:, :, :, DK:DK + 1], 1.0)

    # ---------------- Q^T / K^T projections ----------------
    ev = 0
    # DMA order: low-column halves of xq AND xk first -> first scores matmuls
    # (which need only qt/kt hf0 grains) can start ~14us earlier.
    wts, xts = {}, {}
    for key, wdr, xdr in (("q", wq, xq), ("k", wk, xk)):
        wt, xt = [], []
        for m in range(MT):
            t = wp.tile([128, DG], BF16, tag="w", name="w")
            nc.sync.dma_start(t[:], wdr[m * 128:(m + 1) * 128, :])
            wt.append(t)
            xt.append(xp.tile([128, S], BF16, tag="x", name="x"))
        for m in range(MT):
            nc.sync.dma_start(xt[m][:, 0:HS], xdr[m * 128:(m + 1) * 128, 0:HS])
        wts[key], xts[key] = wt, xt
    nc.sync.dma_start(bt[:], bT[:])
    for key, xdr in (("q", xq), ("k", xk)):
        for m in range(MT):
            nc.sync.dma_start(xts[key][m][:, HS:S],
                              xdr[m * 128:(m + 1) * 128, HS:S])

    for bofs, (key, outsb) in enumerate((("q", qt), ("k", kt))):
        wt, xt = wts[key], xts[key]
        for dch in range(1):
            for hf in range(2):
                psum = ps.tile([128, HS], F32, tag="ps", name="ps")
                for m in range(MT):
                    for qc in range(2):
                        nc.tensor.matmul(
                            psum[:, qc * 512:(qc + 1) * 512],
                            wt[m][:, dch * 128:(dch + 1) * 128],
                            xt[m][:, hf * HS + qc * 512:hf * HS + (qc + 1) * 512],
                            start=(m == 0), stop=False)
                for qc in range(2):
                    nc.tensor.matmul(
                        psum[:, qc * 512:(qc + 1) * 512],
                        bt[:, bofs * DG + dch * 128:bofs * DG + (dch + 1) * 128],
                        ones[:, hf * HS + qc * 512:hf * HS + (qc + 1) * 512],
                        start=False, stop=True)
                nc.vector.tensor_copy(outsb[dch][:, hf * HS:(hf + 1) * HS],
                                      psum[:])

    # ---------------- V projection (natural [s, d] layout) ----------------
    wvt, xvt = [], []
    for m in range(MT):
        t = wp.tile([128, DG], BF16, tag="w", name="w")
        nc.sync.dma_start(t[:], wv[m * 128:(m + 1) * 128, :])
        wvt.append(t)
        xvt.append(xp.tile([128, S], BF16, tag="x", name="x"))
    for hf in range(2):
        for m in range(MT):
            nc.sync.dma_start(
                xvt[m][:, hf * HS:(hf + 1) * HS],
                xv[m * 128:(m + 1) * 128, hf * HS:(hf + 1) * HS])
    def vproj_grain(sc):
        vps = smallps.tile([128, NH, DK], F32, tag="sm", name="vps")
        for m in range(MT):
            nc.tensor.matmul(
                vps[:, :, :],
                xvt[m][:, sc * 128:(sc + 1) * 128],
                wvt[m][:],
                start=(m == 0), stop=False)
        nc.tensor.matmul(
            vps[:, :, :],
            ones[:, sc * 128:(sc + 1) * 128],
            bt[:, 2 * DG:3 * DG],
            start=False, stop=True)
        nc.vector.tensor_copy(vaug[:, sc, :, 0:DK], vps[:, :, :])

    # out-projection weights (DMA sits behind the x tiles; needed much later)
    wot = []
    for dch in range(2):
        t = wop.tile([128, D], BF16, tag="wo", name="wo")
        nc.sync.dma_start(t[:], wo[dch * 128:(dch + 1) * 128, :])
        wot.append(t)

    # ---------------- attention per head ----------------
    def outproj_grain(dch, oc, qp, outT, ev):
        """A [128, 1024] out-projection pair: two 512-wide psum grains,
        evicted into one SBUF tile, shipped with a single DMA."""
        osb = outp.tile([128, HS], BF16, tag="out", name="out")
        for j in range(2):
            q4 = 2 * qp + j
            ops = smallps.tile([128, 512], F32, tag="sm", name="ops")
            nc.tensor.matmul(
                ops[:], wot[dch][:, oc * 128:(oc + 1) * 128],
                ctxn[dch][:, q4 * 512:(q4 + 1) * 512],
                start=True, stop=True)
            nc.vector.tensor_copy(osb[:, j * 512:(j + 1) * 512], ops[:])
        nc.sync.dma_start(
            outT[oc * 128:(oc + 1) * 128, qp * HS:(qp + 1) * HS], osb[:])

    # out-grains of finished ctxn regions are interleaved into later heads
    from collections import deque
    pending = deque()
    for h in range(NH):
        dch, po = h // 2, 64 * (h % 2)
        for hf in range(2):          # q-pass split: ctx only [65, 1024] psum
            ctx = ctxps.tile([DK + 1, HS], F32, tag="ctx", name="ctx")
            atts = {}
            LAG = 6   # emit PV L chunks behind scores: next L scores outrank it
            for cc in range(KC + LAG):
                if cc < KC:
                    c = cc
                    if h == 0 and hf == 0:
                        vproj_grain(c)
                    scs = ps.tile([128, HS], F32, tag="ps", name="ps")
                    for qc in range(2):
                        nc.tensor.matmul(
                            scs[:, qc * 512:(qc + 1) * 512],
                            kt[dch][po:po + DK, c * 128:(c + 1) * 128],
                            qt[dch][po:po + DK,
                                    hf * HS + qc * 512:hf * HS + (qc + 1) * 512],
                            start=True, stop=True)
                    att = attnp.tile([128, HS], BF16, tag="attn", name="attn")
                    nc.scalar.activation(att[:], scs[:],
                                         mybir.ActivationFunctionType.Exp,
                                         scale=0.125)
                    atts[c] = att
                if cc >= LAG:
                    c = cc - LAG
                    att = atts.pop(c)
                    for qc in range(2):
                        nc.tensor.matmul(
                            ctx[:, qc * 512:(qc + 1) * 512],
                            vaug[:, c, h, :],
                            att[:, qc * 512:(qc + 1) * 512],
                            start=(c == 0), stop=(c == KC - 1))
                if pending and cc % 2 == 1:
                    outproj_grain(*pending.popleft())
            cp = zp.tile([DK + 1, HS], F32, tag="cp", name="cp")
            nc.vector.tensor_copy(cp[:], ctx[:])  # frees the ctx psum slot fast
            zr = zp.tile([1, HS], F32, tag="zr", name="zr")
            nc.vector.reciprocal(zr[:], cp[DK:DK + 1, :])
            bc = zp.tile([DK, HS], F32, tag="bc", name="bc")
            nc.gpsimd.partition_broadcast(bc[:], zr[:])
            nc.vector.tensor_mul(ctxn[dch][po:po + DK, hf * HS:(hf + 1) * HS],
                                 cp[0:DK, :], bc[:])
            if h == 3 and hf == 0:
                # ctxn[1][:, 0:HS] complete -> its 8 pairs can go
                pending.extend((1, oc, 0, outT1, 0) for oc in range(8))

        if h == 1:
            pending.extend((0, oc, qp, outT0, 0)
                           for oc in range(8) for qp in range(2))
            # deferred dch1 Q/K projections (needed by heads 2/3 only):
            # re-DMA x into fresh tiles (queue is idle now), small psum grains
            for bofs, (key, xdr, outsb) in enumerate(
                    (("q", xq, qt), ("k", xk, kt))):
                wt = wts[key]
                xt2 = []
                for m in range(MT):
                    t = xp.tile([128, S], BF16, tag="x", name="x2")
                    nc.sync.dma_start(t[:], xdr[m * 128:(m + 1) * 128, :])
                    xt2.append(t)
                for hf2 in range(2):
                    for qc in range(2):
                        psum = smallps.tile([128, 512], F32, tag="sm",
                                            name="ps2")
                        for m in range(MT):
                            nc.tensor.matmul(
                                psum[:],
                                wt[m][:, 128:256],
                                xt2[m][:, hf2 * HS + qc * 512:
                                       hf2 * HS + (qc + 1) * 512],
                                start=(m == 0), stop=False)
                        nc.tensor.matmul(
                            psum[:],
                            bt[:, bofs * DG + 128:bofs * DG + 256],
                            ones[:, hf2 * HS + qc * 512:
                                 hf2 * HS + (qc + 1) * 512],
                            start=False, stop=True)
                        nc.vector.tensor_copy(
                            outsb[1][:, hf2 * HS + qc * 512:
                                     hf2 * HS + (qc + 1) * 512], psum[:])
    # tail: whatever pairs remain, plus the dch1 upper-half pass
    pending.extend((1, oc, 1, outT1, oc) for oc in range(8))
    for g in pending:
        outproj_grain(*g)


def build_nc(reps=1):
    nc = bacc.Bacc("TRN2", target_bir_lowering=False)
    dram = (
        nc.dram_tensor("xq", [D, S], BF16, kind="ExternalInput"),
        nc.dram_tensor("xk", [D, S], BF16, kind="ExternalInput"),
        nc.dram_tensor("xv", [D, S], BF16, kind="ExternalInput"),
        nc.dram_tensor("wq", [D, DG], BF16, kind="ExternalInput"),
        nc.dram_tensor("wk", [D, DG], BF16, kind="ExternalInput"),
        nc.dram_tensor("wv", [D, DG], BF16, kind="ExternalInput"),
        nc.dram_tensor("bT", [1, 3 * DG], BF16, kind="ExternalInput"),
        nc.dram_tensor("wo", [DG, D], BF16, kind="ExternalInput"),
        nc.dram_tensor("outT0", [D, S], BF16, kind="ExternalOutput"),
        nc.dram_tensor("outT1", [D, S], BF16, kind="ExternalOutput"),
    )

    with tile.TileContext(nc) as tc:
        with (
            tc.tile_pool(name="persist", bufs=1) as persist,
            tc.tile_pool(name="xp", bufs=16) as xp,
            tc.tile_pool(name="wp", bufs=26) as wp,
            tc.tile_pool(name="wop", bufs=2) as wop,
            tc.tile_pool(name="attnp", bufs=20) as attnp,
            tc.tile_pool(name="zp", bufs=2) as zp,
            tc.tile_pool(name="outp", bufs=6) as outp,
            tc.tile_pool(name="ps", bufs=2, space="PSUM") as ps,
            tc.tile_pool(name="ctxps", bufs=1, space="PSUM") as ctxps,
            tc.tile_pool(name="smallps", bufs=2, space="PSUM") as smallps,
        ):
            pools = (persist, xp, wp, wop, attnp, zp, outp, ps, ctxps, smallps)
            if reps == 1:
                _emit(nc, pools, dram)
            else:
                with tc.For_i(0, reps, 1):
                    _emit(nc, pools, dram)
    nc.compile()
    return nc


def make_in_maps(query, key, value, Wq, bq, Wk, bk, Wv, bv, Wo, bo):
    bf = ml_dtypes.bfloat16
    query, key, value = (np.asarray(a, np.float32) for a in (query, key, value))
    Wq, bq, Wk, bk, Wv, bv, Wo, bo = (
        np.asarray(a, np.float32) for a in (Wq, bq, Wk, bk, Wv, bv, Wo, bo))
    in_maps = []
    for c in range(N_CORES):
        b, g = divmod(c, 4)
        sl = slice(g * DG, (g + 1) * DG)

        def xa(x):
            return np.ascontiguousarray(x[b].T).astype(bf)

        def wa(W):
            return np.ascontiguousarray(W[sl, :].T).astype(bf)

        in_maps.append({
            "xq": xa(query), "xk": xa(key), "xv": xa(value),
            "wq": wa(Wq), "wk": wa(Wk), "wv": wa(Wv),
            "bT": np.concatenate([bq[sl], bk[sl], bv[sl]])[None, :].astype(bf),
            "wo": np.ascontiguousarray(Wo[:, sl].T).astype(bf),
        })
    return in_maps


_NC_CACHE = {}


def kernel(query, key, value, Wq, bq, Wk, bk, Wv, bv, Wo, bo):
    in_maps = make_in_maps(query, key, value, Wq, bq, Wk, bk, Wv, bv, Wo, bo)
    if 1 not in _NC_CACHE:
        _NC_CACHE[1] = build_nc(1)
    nc = _NC_CACHE[1]
    res = run_bass_kernel_spmd(nc, in_maps, core_ids=list(range(N_CORES)))
    out = np.zeros((2, S, D), np.float32)
    for c in range(N_CORES):
        b = c // 4
        out[b] += np.asarray(res.results[c]["outT0"], np.float32).T
        out[b] += np.asarray(res.results[c]["outT1"], np.float32).T
    out += np.asarray(bo, np.float32)[None, None, :]
    return out

